# revision 34
# baseline (speedup 1.0000x reference)
"""Trainium2 Bass kernel for nn_DKAModule (dynamic-kernel attention), v4.

Data-parallel over B*n = 8192 tokens -> TPC=1024 per core (+10-token halo).
All matmuls bf16 (1 cycle/col on PE at 2.4GHz when back-to-back).

Per core, software-pipelined over heads (group g runs stage1 of head g,
band matmuls of head g-1, diag/chain tail of head g-2):

  stage1:  xp_m = W_in-block^T @ x^T + b_in     (PE; Act evac to bf16)
           3 chunks of 348 cols so every matmul stream hides LDWEIGHTS.
  band:    xtd  = 10-tile DMA transpose of xp_h (1 trigger, SP)
           ps_s = per-128-token-tile banded-conv matmuls (PE: C + L/R
                  halo slices, PSUM-accumulated).  Emission is
                  stationary-grouped [L0][L1 C0][C1 R0][R1] so each
                  xtd weight tile is loaded once per pair and the tiny
                  halo matmuls stream back-to-back behind the 512-col
                  C streams.  Per-token coefficients c (= alpha *
                  x_proj_h @ Wc) are folded into the band matrices ON
                  HOST, so ps_s = S*c already.
           cs   = plain PSUM evac (Act copy, bf16)
  chains:  static conv taps split across engines by measured cost:
           PE diag matmuls (0.43ns/col) for some k=11 taps, DVE STT
           chains (1.04ns/col, f32 accumulator - no 7-tap cap), Pool
           STT chains (idle engine) for the k=21 tails.
  tail:    ps_o = sum_r diag(V_r) @ cs_r (+ PE static taps)  (PE)
           o_h  = DVE STT: ps_o * 1 + chain   (fused evac+merge, bf16)
  stage4:  out  = o^T-blocks @ W_out^T (+ b_out) (PE; Act evac bf16,
           DMA out, host converts to f32)

PE clock-gate (HAM): the PE runs at 1.2GHz until it has been busy for a
full ~3.4us window, and re-throttles after any ~3.4us idle gap.  The
kernel front-loads a dummy-matmul fill so the gate opens at ~4us, and
the DMA schedule keeps PE gaps short.
"""
import sys
import types

import ml_dtypes
import numpy as np

BF16 = ml_dtypes.bfloat16
FP8 = ml_dtypes.float8_e4m3

KS = [3, 3, 7, 7, 11, 11, 21, 21]
H, DM, DH, R, B, N = 8, 1024, 128, 4, 2, 4096
NC = 8
TPC = B * N // NC
PAD = 10
LP = 128  # left zero-pad columns in xp
XF = 1280  # padded xp width = 10 transpose tiles
NT = TPC // 128  # 8 token tiles
TH = TPC + 2 * PAD  # 1044 valid x columns
HEADS = (6, 7, 4, 5, 2, 3, 1, 0)  # k=21 first (long DVE chains), k=3 last
S1CH = [(0, 394), (394, 394), (788, 256)]
# xT is shipped chunk-major: [128, (chunk, i, chunk_cols)] so each chunk
# is one contiguous-row DMA that lands early and covers all 8 blocks
CH_OFF = [0, H * 394, H * 788]
N_WARM = 24  # dummy matmuls: open the PE clock gate + cover DMA lead-in

# static-tap split: PE diag matmuls take the first j0 taps, DVE STT chains
# the rest.  Pool is NOT used for chains: GpSimd tensor ops share SBUF
# ports with the DVE and halve its throughput while active.
# Chains accumulate in bf16 (STT runs 1x either way, but bf16 keeps the
# merge tensor_tensor ops in the DVE 2x performance mode), as two
# interleaved sub-chains per head so consecutive STTs are independent
# (a single in-place chain serializes on DVE writeback latency).
# k=3 heads fully on PE: no DVE chain at the pipeline tail, so stage4
# is not gated on a DVE drain
PE_TAPS = {0: 3, 1: 3, 4: 8, 5: 8, 6: 4, 7: 4}
ACT_TAPS = {}  # Act queue must stay lean: its evacs gate the transposes

# fp8 DoubleRow vdiag: cs is evacuated as fp8e4 scaled by 2^CS_SH, V is
# packed in rank pairs as fp8 scaled by 2^VD_SH; the tail evac rescales
# by 2^-(CS_SH+VD_SH).  gdiag is pre-scaled by the same amount on host.
# The dynamic path is ~0.6% of the output's magnitude, so fp8's ~3%
# relative error contributes ~2e-4 of output scale.
BAND_SH = 7  # band ships as fp8e4 scaled 2^BAND_SH (halves the 9MB DMA)
CS_SH = 7  # equals BAND_SH: cs inherits the band scaling through the matmul
VD_SH = 5


def _dve_taps(h):
    return PE_TAPS.get(h, 0), ACT_TAPS.get(h, (KS[h],))[0]


_MODULE_CACHE = {}


def _install_ntff_hook_shim():
    """This image's antenv lacks axon_hooks; provide it so profiling works."""
    if "antenv.axon_hooks" in sys.modules:
        return
    try:
        from trn_agent_boot.trn_boot import _ntff_profile_via_ctypes

        hook = _ntff_profile_via_ctypes("/opt/axon/libaxon_pjrt.so")
    except Exception:
        hook = None
    mod = types.ModuleType("antenv.axon_hooks")
    mod.get_axon_ntff_profile_hook = lambda: hook
    mod.set_axon_ntff_profile_hook = lambda h: None
    sys.modules["antenv.axon_hooks"] = mod


def _split_multi_waits(nc, mybir):
    """walrus codegen allows a single sync-wait per instruction; hoist
    extras onto a chain of single-wait NoOps on the same engine."""
    for f in nc.m.functions:
        for blk in f.blocks:
            new_insts = []
            for inst in blk.instructions:
                si = getattr(inst, "sync_info", None)
                ow = list(si.on_wait) if si and si.on_wait else []
                if len(ow) >= 2:
                    for i, w in enumerate(ow[:-1]):
                        new_insts.append(
                            mybir.InstNoOp(
                                name=f"{inst.name}-wn{i}",
                                ins=[],
                                outs=[],
                                engine=inst.engine,
                                sync_info=mybir.SyncInfo(on_wait=[w], on_update=[]),
                            )
                        )
                    inst.sync_info = mybir.SyncInfo(
                        on_wait=[ow[-1]],
                        on_update=list(si.on_update) if si.on_update else [],
                    )
                new_insts.append(inst)
            blk.instructions = new_insts


def _tile_cols(h):
    """Band cols per token-tile for head h: C (R*128) + L (R*p) + R (R*p)."""
    p = KS[h] // 2
    return R * (128 + 2 * p)


def _band_off(h):
    """Column offset of head h's packed per-tile band blocks."""
    off = 0
    for g in range(h):
        off += NT * _tile_cols(g)
    return off


BAND_TOTAL = _band_off(H - 1) + NT * _tile_cols(H - 1)

GD_OFF = {}
_o = 0
for _h in sorted(PE_TAPS):
    GD_OFF[_h] = _o
    _o += PE_TAPS[_h] * DH
GD_TOTAL = _o


def _build_module(has_bias):
    import concourse.bass as bass
    import concourse.tile as tile
    from concourse import mybir

    f32 = mybir.dt.float32
    bf16 = mybir.dt.bfloat16
    MULT = mybir.AluOpType.mult
    ADD = mybir.AluOpType.add
    IDENT = mybir.ActivationFunctionType.Identity

    nc = bass.Bass(trn_type="TRN2")

    # host ships these pre-interleaved in SBUF layout: contiguous 2KB+
    # rows keep HWDGE descriptor counts at 128/transfer (a [p, i, c]
    # block-strided AP costs ~1024 descriptors = ~3-11us of engine time)
    xT_d = nc.dram_tensor("xT", [128, H * TH], bf16, kind="ExternalInput")
    w_inT_d = nc.dram_tensor("w_inT", [128, H * DM], bf16, kind="ExternalInput")
    w_outT_d = nc.dram_tensor("w_outT", [128, H * DM], bf16, kind="ExternalInput")
    band_d = nc.dram_tensor(
        "band", [128, BAND_TOTAL], mybir.dt.float8e4, kind="ExternalInput"
    )
    vdiag_d = nc.dram_tensor(
        "vdiag", [DH, H * R * DH], mybir.dt.float8e4, kind="ExternalInput"
    )
    gdiag_d = nc.dram_tensor("gdiag", [DH, GD_TOTAL], bf16, kind="ExternalInput")
    gvec_d = nc.dram_tensor("gvec", [DH, H * 21], f32, kind="ExternalInput")
    b_in_d = nc.dram_tensor("b_in", [128, H], f32, kind="ExternalInput")
    if has_bias:
        b_out_d = nc.dram_tensor("b_out", [1, DM], bf16, kind="ExternalInput")
    out_d = nc.dram_tensor("out", [TPC, DM], bf16, kind="ExternalOutput")


    with tile.TileContext(nc) as tc:
        with tc.tile_pool(name="const", bufs=1) as pc:
            xp_sb = [pc.tile([DH, XF], bf16, name=f"xp{m}") for m in range(H)]
            o_sb = [pc.tile([DH, TPC], bf16, name=f"o{h}") for h in range(H)]
            w_all = pc.tile([128, H * DM], bf16, name="w_all")
            xT_all = pc.tile([128, H * TH], bf16, name="xT_all")
            wo_all = pc.tile([128, H * DM], bf16, name="wo_all")
            gvec_sb = pc.tile([DH, H * 21], f32, name="gvec_sb")
            vd_sb = pc.tile([DH, H * R * DH], mybir.dt.float8e4, name="vd_sb")
            if GD_TOTAL:
                gd_sb = pc.tile([DH, GD_TOTAL], bf16, name="gd_sb")
            b_in_sb = pc.tile([128, H], f32, name="b_in_sb")
            if has_bias:
                ones_sb = pc.tile([1, 128], bf16, name="ones_sb")
                bo_sb = pc.tile([1, DM], bf16, name="bo_sb")
                nc.gpsimd.memset(ones_sb, 1.0)
                nc.scalar.dma_start(out=bo_sb, in_=b_out_d[:, :])


            for m in range(H):
                nc.gpsimd.memset(xp_sb[m][:, 0 : LP - PAD], 0)
                nc.gpsimd.memset(xp_sb[m][:, LP + TPC + PAD : XF], 0)

            # warm the activation table off the critical path
            warm = pc.tile([1, 2], f32, name="warm")
            nc.gpsimd.memset(warm, 0)
            nc.scalar.activation(
                out=warm[:, 1:2], in_=warm[:, 0:1], func=IDENT, bias=0.0, scale=1.0
            )

            # ---- preamble DMAs, first-needed first ----
            # w is m-major [128, (m, i*128)], xT is [128, (i, c)]; all
            # transfers are contiguous-row (128 descriptors).  Act queue:
            # b_in + w blocks; SP queue: xT halves + transposes; Pool
            # queue: bands + cold weights.
            m0, m1 = HEADS[0], HEADS[1]
            MB = 8 * 128  # cols per m-block in w_all
            nc.scalar.dma_start(
                out=w_all[:, m0 * MB : (m0 + 1) * MB],
                in_=w_inT_d[:, m0 * MB : (m0 + 1) * MB],
            )
            # xT rides the Act queue: q1 (SP) stays clear for the
            # latency-critical per-group transposes
            def xt_chunk(ci, eng):
                o0, cn = CH_OFF[ci], S1CH[ci][1]
                eng.dma_start(
                    out=xT_all[:, o0 : o0 + H * cn], in_=xT_d[:, o0 : o0 + H * cn]
                )

            # one xT chunk per DMA queue so they land in parallel
            xt_chunk(0, nc.scalar)
            xt_chunk(1, nc.sync)
            xt_chunk(2, nc.gpsimd)
            nc.scalar.dma_start(out=b_in_sb, in_=b_in_d[:, :])
            nc.scalar.dma_start(
                out=w_all[:, m1 * MB : (m1 + 1) * MB],
                in_=w_inT_d[:, m1 * MB : (m1 + 1) * MB],
            )
            nc.scalar.dma_start(out=gvec_sb, in_=gvec_d[:, :])

            with tc.tile_pool(name="ps1", bufs=2, space="PSUM") as pp1, tc.tile_pool(
                name="ps3", bufs=2, space="PSUM"
            ) as pp3, tc.tile_pool(
                name="pso", bufs=2, space="PSUM"
            ) as pp_o, tc.tile_pool(name="pband", bufs=3) as p_band, tc.tile_pool(
                name="pcs", bufs=3
            ) as p_cs, tc.tile_pool(name="pxtd", bufs=3) as p_xtd, tc.tile_pool(
                name="pchain", bufs=2
            ) as p_ch, tc.tile_pool(name="pat", bufs=4) as p_at:
                band_tiles = {}
                cs_tiles = {}
                xtd_tiles = {}
                chain_tiles = {}
                pool_tiles = {}

                def issue_band_dma(h, split=False):
                    boff = _band_off(h)
                    bw = NT * _tile_cols(h)
                    bt = p_band.tile(
                        [128, bw], mybir.dt.float8e4, name=f"band{h}", tag="band"
                    )
                    if split:
                        # first head: land the first pairs' blocks early
                        hw_ = 4 * _tile_cols(h)
                        nc.gpsimd.dma_start(
                            out=bt[:, :hw_], in_=band_d[:, boff : boff + hw_]
                        )
                        nc.gpsimd.dma_start(
                            out=bt[:, hw_:], in_=band_d[:, boff + hw_ : boff + bw]
                        )
                    else:
                        # Pool-issued DMA: separate queue, no FIFO conflict
                        # with the JIT transposes (SP) or weights (Act)
                        nc.gpsimd.dma_start(out=bt, in_=band_d[:, boff : boff + bw])
                    band_tiles[h] = bt

                def s1_chunk(m, ci):
                    c0, cn = S1CH[ci]
                    # 512-wide f32 tile = exactly one PSUM bank (zero-region
                    # aligned); only the first cn cols are used
                    ps1 = pp1.tile([128, 512], f32, name="ps1", tag="ps1")
                    o0 = CH_OFF[ci]
                    for i in range(H):
                        wb = (m * H + i) * 128
                        nc.tensor.matmul(
                            ps1[:, :cn],
                            w_all[:, wb : wb + 128],
                            xT_all[:, o0 + i * cn : o0 + (i + 1) * cn],
                            start=(i == 0),
                            stop=(i == H - 1),
                        )
                    nc.scalar.activation(
                        out=xp_sb[m][:, LP - PAD + c0 : LP - PAD + c0 + cn],
                        in_=ps1[:, :cn],
                        func=IDENT,
                        bias=b_in_sb[:, m : m + 1],
                        scale=1.0,
                    )

                def issue_transpose_0(m):
                    # tiles 0..6 (xp cols 0:896) — gated on s1 chunks 0-1;
                    # covers band pairs 0..2 of the next group
                    xtd = p_xtd.tile([128, XF // 128, 128], bf16, name="xtd", tag="xtd")
                    nc.sync.dma_start_transpose(
                        out=xtd[:, 0:7, :], in_=xp_sb[m][:, 0:896]
                    )
                    xtd_tiles[m] = xtd

                def issue_transpose_2(m):
                    nc.sync.dma_start_transpose(
                        out=xtd_tiles[m][:, 7:10, :], in_=xp_sb[m][:, 896:XF]
                    )

                def chain_cont(eng, tile_out, taps):
                    for in0, sc in taps:
                        eng.scalar_tensor_tensor(
                            out=tile_out,
                            in0=in0,
                            scalar=sc,
                            in1=tile_out,
                            op0=MULT,
                            op1=ADD,
                        )

                def chain(eng, tile_out, taps):
                    in0, sc = taps[0]
                    eng.tensor_scalar(
                        out=tile_out, in0=in0, scalar1=sc, scalar2=None, op0=MULT
                    )
                    chain_cont(eng, tile_out, taps[1:])

                def band_pair(h, pair):
                    """Band matmuls for token tiles 2*pair, 2*pair+1 into one
                    2-bank PSUM tile + a single paired Act evac.  Emission is
                    stationary-grouped so each xtd tile is loaded once."""
                    k = KS[h]
                    p = k // 2
                    tcols = _tile_cols(h)
                    bt = band_tiles[h]
                    xtd = xtd_tiles[h]
                    if pair == 0:
                        cs = p_cs.tile(
                            [128, R, TPC], mybir.dt.float8e4, name=f"cs{h}", tag="cs"
                        )
                        cs_tiles[h] = cs
                    cs = cs_tiles[h]
                    psp = pp3.tile([128, 2, R, 128], f32, name="ps_s", tag="ps_s")

                    def blocks(b):
                        o = b * tcols
                        bC = bt[:, o : o + R * 128].rearrange("q (r w) -> q r w", r=R)
                        bL = bt[:, o + R * 128 : o + R * 128 + R * p].rearrange(
                            "q (r w) -> q r w", r=R
                        )
                        bR = bt[:, o + R * 128 + R * p : o + tcols].rearrange(
                            "q (r w) -> q r w", r=R
                        )
                        return bC, bL, bR

                    b0 = 2 * pair
                    b1 = b0 + 1
                    C0, L0, R0 = blocks(b0)
                    C1, L1, R1 = blocks(b1)
                    ps0 = psp[:, 0]
                    ps1_ = psp[:, 1]
                    # stationary sequence: b0 | b0+1 (L1, C0) | b0+2 (C1, R0) | b0+3
                    nc.tensor.matmul(
                        ps0[:, :, 0:p], xtd[:, b0, :], L0, start=True, stop=False
                    )
                    nc.tensor.matmul(
                        ps1_[:, :, 0:p], xtd[:, b1, :], L1, start=True, stop=False
                    )
                    nc.tensor.matmul(ps0, xtd[:, b0 + 1, :], C0, start=False, stop=False)
                    nc.tensor.matmul(
                        ps1_, xtd[:, b1 + 1, :], C1, start=False, stop=False
                    )
                    nc.tensor.matmul(
                        ps0[:, :, 128 - p : 128],
                        xtd[:, b0 + 2, :],
                        R0,
                        start=False,
                        stop=True,
                    )
                    nc.tensor.matmul(
                        ps1_[:, :, 128 - p : 128],
                        xtd[:, b1 + 2, :],
                        R1,
                        start=False,
                        stop=True,
                    )
                    nc.scalar.copy(
                        cs[:, :, b0 * 128 : (b0 + 2) * 128],
                        psp.rearrange("q b r w -> q r b w"),
                    )
                    if pair == 3:
                        band_tiles.pop(h)
                        xtd_tiles.pop(h)

                def tap(h, j):
                    p = KS[h] // 2
                    return (
                        xp_sb[h][:, LP + j - p : LP + j - p + TPC],
                        gvec_sb[:, h * 21 + j : h * 21 + j + 1],
                    )

                def emit_chain_steps(steps, n):
                    for _ in range(n):
                        eng_op = steps.pop(0)
                        eng_op()

                def build_chain_steps(h, tiles, taps):
                    # round-robin taps over the sub-chain tiles; consecutive
                    # DVE ops then target different tiles and pipeline
                    nt = len(tiles)
                    steps = []
                    for i, (in0, sc) in enumerate(taps):
                        t = tiles[i % nt]
                        if i < nt:
                            steps.append(
                                lambda t=t, in0=in0, sc=sc: nc.vector.tensor_scalar(
                                    out=t, in0=in0, scalar1=sc, scalar2=None, op0=MULT
                                )
                            )
                        else:
                            steps.append(
                                lambda t=t, in0=in0, sc=sc: nc.vector.scalar_tensor_tensor(
                                    out=t,
                                    in0=in0,
                                    scalar=sc,
                                    in1=t,
                                    op0=MULT,
                                    op1=ADD,
                                )
                            )
                    return steps

                def band_chains(h):
                    # static MAC chains for taps not on PE, split: first half
                    # here, second half in tail_stage's group so long chains
                    # don't pin the tail's PSUM banks for a whole group.
                    # ACT_TAPS are computed as Act per-partition-scaled
                    # multiplies; the DVE only pays a 2x-rate add for them.
                    j0, j1 = _dve_taps(h)
                    taps = [tap(h, j) for j in range(j0, j1)]
                    if not taps:
                        return
                    tiles = [p_ch.tile([DH, TPC], bf16, name=f"ch{h}", tag="chA")]
                    if len(taps) >= 6:
                        tiles.append(
                            p_ch.tile([DH, TPC], bf16, name=f"ch{h}b", tag="chB")
                        )
                    steps = build_chain_steps(h, tiles, taps)
                    if h in ACT_TAPS:
                        q0, q1 = ACT_TAPS[h]
                        asteps = []
                        for qi, j in enumerate(range(q0, q1)):
                            in0, gc = tap(h, j)
                            at = p_at.tile([DH, TPC], bf16, name=f"at{h}", tag="at")
                            nc.scalar.mul(at, in0, gc)
                            t = tiles[(len(taps) + qi) % len(tiles)]
                            asteps.append(
                                lambda t=t, at=at: nc.vector.tensor_tensor(
                                    t, t, at, op=ADD
                                )
                            )
                        # act-tap adds run after the sub-chain inits
                        # (the first step of each tile overwrites it)
                        steps = steps + asteps
                    emit_chain_steps(steps, (len(steps) + 1) // 2)
                    chain_tiles[h] = (tiles, steps)

                def tail_stage(h):
                    cs = cs_tiles.pop(h)
                    k = KS[h]
                    p = k // 2
                    j0 = PE_TAPS.get(h, 0)
                    sacc = None
                    if h in chain_tiles:
                        tiles, steps = chain_tiles.pop(h)
                        emit_chain_steps(steps, len(steps))
                        if len(tiles) == 2:
                            nc.vector.tensor_tensor(tiles[0], tiles[0], tiles[1], op=ADD)
                        sacc = tiles[0]
                    tmp_o = p_ch.tile([DH, TPC], bf16, name="tmp_o", tag="tmpo")
                    for ci, c0 in enumerate((0, 512)):
                        ps_o = pp_o.tile([128, 512], f32, name="ps_o", tag="ps_o")
                        n_mm = R // 2 + j0
                        idx = 0
                        for q in range(R // 2):
                            # fp8 DoubleRow: ranks (2q, 2q+1) in one matmul
                            vq = vd_sb[
                                :, (h * R + 2 * q) * DH : (h * R + 2 * q + 2) * DH
                            ].rearrange("d (i e) -> d i e", i=2)
                            nc.tensor.matmul(
                                ps_o,
                                vq,
                                cs[:, 2 * q : 2 * q + 2, c0 : c0 + 512],
                                start=(idx == 0),
                                stop=(idx == n_mm - 1),
                                perf_mode=mybir.MatmulPerfMode.DoubleRow,
                            )
                            idx += 1
                        go = GD_OFF.get(h, 0)
                        for j in range(j0):
                            nc.tensor.matmul(
                                ps_o,
                                gd_sb[:, go + j * DH : go + (j + 1) * DH],
                                xp_sb[h][:, LP + j - p + c0 : LP + j - p + c0 + 512],
                                start=False,
                                stop=(idx == n_mm - 1),
                            )
                            idx += 1
                        # fast Act evac (rescales the 2^(CS_SH+VD_SH) fp8
                        # scaling away) so the PSUM bank frees quickly; the
                        # bf16 chain merge then runs at DVE 2x rate
                        dst = tmp_o if sacc is not None else o_sb[h]
                        nc.scalar.mul(
                            dst[:, c0 : c0 + 512], ps_o, 2.0 ** -(CS_SH + VD_SH)
                        )
                    if sacc is not None:
                        nc.vector.tensor_tensor(o_sb[h], tmp_o, sacc, op=ADD)

                # ---------------- pipelined emission ----------------
                # PE dummy-fill on zeroed data: opens the HAM clock gate and
                # keeps PE busy while the first input DMAs land
                wscr = pc.tile([128, 512], bf16, name="wscr")
                nc.vector.memset(wscr, 0)
                for _ in range(N_WARM):
                    psw = pp1.tile([128, 512], f32, name="ps1", tag="ps1")
                    nc.tensor.matmul(
                        psw, wscr[:, 0:128], wscr, start=True, stop=True
                    )

                issue_band_dma(HEADS[0], split=True)
                # rest of w_in (m-blocks 0..5; m0, m1 are 6 and 7)
                nc.gpsimd.dma_start(
                    out=w_all[:, : 6 * MB], in_=w_inT_d[:, : 6 * MB]
                )
                nc.gpsimd.dma_start(out=vd_sb, in_=vdiag_d[:, :])
                if GD_TOTAL:
                    nc.gpsimd.dma_start(out=gd_sb, in_=gdiag_d[:, :])
                for gi, m in enumerate(HEADS):
                    hp = HEADS[gi - 1] if gi >= 1 else None  # band stage
                    hq = HEADS[gi - 2] if gi >= 2 else None  # tail stage
                    if gi == 3:
                        nc.gpsimd.dma_start(out=wo_all, in_=w_outT_d[:, :])
                    s1_chunk(m, 0)
                    if hp is not None:
                        band_pair(hp, 0)
                    s1_chunk(m, 1)
                    issue_transpose_0(m)
                    if hp is not None:
                        band_pair(hp, 1)
                    s1_chunk(m, 2)
                    issue_transpose_2(m)
                    # next head's band lands behind this group's transpose on
                    # the SP queue: ~1.5 groups of lead, transpose not delayed
                    if gi + 1 < H:
                        issue_band_dma(HEADS[gi + 1])
                    if hp is not None:
                        band_pair(hp, 2)
                        band_pair(hp, 3)
                    if hq is not None:
                        tail_stage(hq)
                    if hp is not None:
                        band_chains(hp)
                for pair in range(4):
                    band_pair(HEADS[7], pair)
                tail_stage(HEADS[6])
                band_chains(HEADS[7])
                tail_stage(HEADS[7])

            # ---------------- stage 4: out projection ----------------
            with tc.tile_pool(name="ps4", bufs=4, space="PSUM") as pp4, tc.tile_pool(
                name="post", bufs=4
            ) as p_ost:
                for t in range(NT):
                    for ei, e0 in enumerate((0, 512)):
                        ps4 = pp4.tile([128, 512], f32, name="ps4", tag="ps4")
                        n_mm = H + (1 if has_bias else 0)
                        for i in range(H):
                            nc.tensor.matmul(
                                ps4,
                                o_sb[i][:, t * 128 : (t + 1) * 128],
                                wo_all[:, i * DM + e0 : i * DM + e0 + 512],
                                start=(i == 0),
                                stop=(i == n_mm - 1),
                            )
                        if has_bias:
                            nc.tensor.matmul(
                                ps4,
                                ones_sb,
                                bo_sb[:, e0 : e0 + 512],
                                start=False,
                                stop=True,
                            )
                        ost = p_ost.tile([128, 512], bf16, name="ost", tag="ost")
                        nc.vector.tensor_scalar(
                            out=ost, in0=ps4, scalar1=1.0, scalar2=None, op0=MULT
                        )
                        eng = nc.sync
                        eng.dma_start(
                            out=out_d[t * 128 : (t + 1) * 128, e0 : e0 + 512],
                            in_=ost,
                        )

    _split_multi_waits(nc, mybir)
    return nc


def _band_bases(A):
    """Per-head unscaled band blocks (f32): C (128,R,128), L/R (128,R,p)."""
    bases = []
    t = np.arange(128)[:, None]
    for h in range(H):
        k = KS[h]
        p = k // 2
        w = np.arange(128)[None, :]
        dC = t - w
        mC = np.abs(dC) <= p
        iC = np.clip(dC + p, 0, k - 1)
        wl = np.arange(p)[None, :] if p else np.zeros((1, 0), int)
        dL = t - wl - 128
        mL = (dL >= -p) & (dL <= p)
        iL = np.clip(dL + p, 0, k - 1)
        u = np.arange(p)[None, :] if p else np.zeros((1, 0), int)
        dR = t + p - u  # t - (128-p+u) + 128
        mR = (dR >= -p) & (dR <= p)
        iR = np.clip(dR + p, 0, k - 1)
        C = np.where(mC[:, None, :], A[h][:, iC].transpose(1, 0, 2), 0.0)
        L = np.where(mL[:, None, :], A[h][:, iL].transpose(1, 0, 2), 0.0)
        Rb = np.where(mR[:, None, :], A[h][:, iR].transpose(1, 0, 2), 0.0)
        bases.append((C, L, Rb))
    return bases


def _host_prep(inputs):
    x = np.ascontiguousarray(np.asarray(inputs["x"], dtype=np.float32))
    W_in = np.asarray(inputs["W_in"], dtype=np.float32)
    b_in = np.asarray(inputs["b_in"], dtype=np.float32)
    W_out = np.asarray(inputs["W_out"], dtype=np.float32)
    b_out = np.asarray(inputs["b_out"], dtype=np.float32)
    Wc = np.asarray(inputs["Wc"], dtype=np.float32)
    A = np.asarray(inputs["A"], dtype=np.float32)
    V = np.asarray(inputs["V"], dtype=np.float32)
    base = np.asarray(inputs["base"], dtype=np.float32)
    alphas = np.asarray(inputs["alphas"], dtype=np.float32)

    alpha = 1.0 / (1.0 + np.exp(-alphas))
    W_inT = np.ascontiguousarray(W_in.T)
    W_outT = np.ascontiguousarray(W_out.T)
    Wc_aug = np.zeros((DM, H * R), dtype=np.float32)
    for h in range(H):
        # alpha folded into c
        Wc_aug[:, R * h : R * h + R] = alpha[h] * (
            W_inT[:, h * DH : (h + 1) * DH] @ Wc[h]
        )

    bases = _band_bases(A)

    gvec = np.zeros((DH, H, 21), dtype=np.float32)
    for h in range(H):
        k = KS[h]
        gvec[:, h, :k] = ((1.0 - alpha[h]) * base[h, :k]).T

    dd = np.arange(DH)
    vd = np.zeros((DH, H, R, DH), dtype=np.float32)
    for h in range(H):
        for r in range(R):
            vd[dd, h, r, dd] = V[h, r]
    gd = np.zeros((DH, max(GD_TOTAL, 1)), dtype=np.float32)
    for h in sorted(PE_TAPS):
        go = GD_OFF[h]
        g = (1.0 - alpha[h]) * base[h, : PE_TAPS[h]]  # (j0, DH)
        for j in range(PE_TAPS[h]):
            gd[dd, go + j * DH + dd] = g[j]

    # pre-interleave into the exact SBUF layouts (contiguous rows keep
    # DMA descriptor counts minimal): w m-major [128, (m, i, 128)],
    # w_out i-major [128, (i, 1024)]
    w_m = (
        W_inT.reshape(H, 128, H, 128).transpose(1, 2, 0, 3).reshape(128, H * DM)
    )
    wo_i = W_outT.reshape(H, 128, DM).transpose(1, 0, 2).reshape(128, H * DM)
    prep = {
        "w_inT": np.ascontiguousarray(w_m).astype(BF16),
        "w_outT": np.ascontiguousarray(wo_i).astype(BF16),
        "vdiag": (vd.reshape(DH, H * R * DH) * 2.0**VD_SH).astype(FP8),
        "gvec": gvec.reshape(DH, H * 21).copy(),
        "b_in": np.ascontiguousarray(b_in.reshape(H, 128).T),
    }
    if GD_TOTAL:
        prep["gdiag"] = (gd[:, :GD_TOTAL] * 2.0 ** (CS_SH + VD_SH)).astype(BF16)
    has_bias = bool(np.any(b_out != 0.0))
    if has_bias:
        prep["b_out"] = b_out.reshape(1, DM).astype(BF16)

    xT_slices = []
    band_slices = []
    per_b = NC // B
    for c in range(NC):
        bb = c // per_b
        s = (c % per_b) * TPC
        sl = np.zeros((TH, DM), dtype=np.float32)
        lo, hi = s - PAD, s + TPC + PAD
        clo, chi = max(lo, 0), min(hi, N)
        sl[clo - lo : chi - lo] = x[bb, clo:chi]
        slT = sl.T.reshape(H, 128, TH).transpose(1, 0, 2)  # [128, i, c]
        xh = np.empty((128, H * TH), dtype=np.float32)
        for ci, (c0, cn) in enumerate(S1CH):
            o0 = CH_OFF[ci]
            xh[:, o0 : o0 + H * cn] = slT[:, :, c0 : c0 + cn].reshape(128, H * cn)
        xT_slices.append(np.ascontiguousarray(xh).astype(BF16))
        cc = (sl[PAD : PAD + TPC] @ Wc_aug).T.reshape(H, R, TPC)  # alpha*c

        band = np.empty((128, BAND_TOTAL), dtype=np.float32)
        for h in range(H):
            k = KS[h]
            p = k // 2
            C, L, Rb = bases[h]
            tcols = _tile_cols(h)
            boff = _band_off(h)
            ch = cc[h]  # (R, TPC)
            for b in range(NT):
                o = boff + b * tcols
                cw = ch[None, :, b * 128 : (b + 1) * 128]  # (1, R, 128)
                band[:, o : o + R * 128] = (C * cw).reshape(128, R * 128)
                if p:
                    cl = ch[None, :, b * 128 : b * 128 + p]
                    band[:, o + R * 128 : o + R * 128 + R * p] = (L * cl).reshape(
                        128, R * p
                    )
                    cr = ch[None, :, (b + 1) * 128 - p : (b + 1) * 128]
                    band[:, o + R * 128 + R * p : o + tcols] = (Rb * cr).reshape(
                        128, R * p
                    )
        band_slices.append((band * 2.0**BAND_SH).astype(FP8))
    return prep, xT_slices, band_slices, has_bias


def _run(inputs, trace=False, **kwargs):
    _install_ntff_hook_shim()
    from concourse.bass_utils import run_bass_kernel_spmd

    prep, xT_slices, band_slices, has_bias = _host_prep(inputs)
    key = ("mod", has_bias)
    if key not in _MODULE_CACHE:
        _MODULE_CACHE[key] = _build_module(has_bias)
    nc = _MODULE_CACHE[key]

    in_maps = []
    for c in range(NC):
        m = dict(prep)
        m["xT"] = xT_slices[c]
        m["band"] = band_slices[c]
        in_maps.append(m)

    res = run_bass_kernel_spmd(
        nc, in_maps, core_ids=list(range(NC)), trace=trace, **kwargs
    )
    outs = [res.results[c]["out"] for c in range(NC)]
    full = np.concatenate(outs, axis=0).reshape(B, N, DM).astype(np.float32)
    return full, res


def kernel(**inputs) -> np.ndarray:
    return _run(inputs)[0]


# revision 35
# speedup vs baseline: 1.1152x; 1.1152x over previous
"""Trainium2 Bass kernel for nn_DKAModule (dynamic-kernel attention), v4.

Data-parallel over B*n = 8192 tokens -> TPC=1024 per core (+10-token halo).
All matmuls bf16 (1 cycle/col on PE at 2.4GHz when back-to-back).

Per core, software-pipelined over heads (group g runs stage1 of head g,
band matmuls of head g-1, diag/chain tail of head g-2):

  stage1:  xp_m = W_in-block^T @ x^T + b_in     (PE; Act evac to bf16)
           3 chunks of 348 cols so every matmul stream hides LDWEIGHTS.
  band:    xtd  = 10-tile DMA transpose of xp_h (1 trigger, SP)
           ps_s = per-128-token-tile banded-conv matmuls (PE: C + L/R
                  halo slices, PSUM-accumulated).  Emission is
                  stationary-grouped [L0][L1 C0][C1 R0][R1] so each
                  xtd weight tile is loaded once per pair and the tiny
                  halo matmuls stream back-to-back behind the 512-col
                  C streams.  Per-token coefficients c (= alpha *
                  x_proj_h @ Wc) are folded into the band matrices ON
                  HOST, so ps_s = S*c already.
           cs   = plain PSUM evac (Act copy, bf16)
  chains:  static conv taps split across engines by measured cost:
           PE diag matmuls (0.43ns/col) for some k=11 taps, DVE STT
           chains (1.04ns/col, f32 accumulator - no 7-tap cap), Pool
           STT chains (idle engine) for the k=21 tails.
  tail:    ps_o = sum_r diag(V_r) @ cs_r (+ PE static taps)  (PE)
           o_h  = DVE STT: ps_o * 1 + chain   (fused evac+merge, bf16)
  stage4:  out  = o^T-blocks @ W_out^T (+ b_out) (PE; Act evac bf16,
           DMA out, host converts to f32)

PE clock-gate (HAM): the PE runs at 1.2GHz until it has been busy for a
full ~3.4us window, and re-throttles after any ~3.4us idle gap.  The
kernel front-loads a dummy-matmul fill so the gate opens at ~4us, and
the DMA schedule keeps PE gaps short.
"""
import sys
import types

import ml_dtypes
import numpy as np

BF16 = ml_dtypes.bfloat16
FP8 = ml_dtypes.float8_e4m3

KS = [3, 3, 7, 7, 11, 11, 21, 21]
H, DM, DH, R, B, N = 8, 1024, 128, 4, 2, 4096
NC = 8
TPC = B * N // NC
PAD = 10
LP = 128  # left zero-pad columns in xp
XF = 1280  # padded xp width = 10 transpose tiles
NT = TPC // 128  # 8 token tiles
TH = TPC + 2 * PAD  # 1044 valid x columns
HEADS = (6, 7, 4, 5, 2, 3, 1, 0)  # k=21 first (long DVE chains), k=3 last
S1CH = [(0, 394), (394, 394), (788, 256)]
# xT is shipped chunk-major: [128, (chunk, i, chunk_cols)] so each chunk
# is one contiguous-row DMA that lands early and covers all 8 blocks
CH_OFF = [0, H * 394, H * 788]
N_WARM = 24  # dummy matmuls: open the PE clock gate + cover DMA lead-in

# static-tap split: PE diag matmuls take the first j0 taps, DVE STT chains
# the rest.  Pool is NOT used for chains: GpSimd tensor ops share SBUF
# ports with the DVE and halve its throughput while active.
# Chains accumulate in bf16 (STT runs 1x either way, but bf16 keeps the
# merge tensor_tensor ops in the DVE 2x performance mode), as two
# interleaved sub-chains per head so consecutive STTs are independent
# (a single in-place chain serializes on DVE writeback latency).
# k=3 heads fully on PE: no DVE chain at the pipeline tail, so stage4
# is not gated on a DVE drain
PE_TAPS = {0: 3, 1: 3, 4: 8, 5: 8, 6: 4, 7: 4}
ACT_TAPS = {}  # Act queue must stay lean: its evacs gate the transposes

# fp8 DoubleRow vdiag: cs is evacuated as fp8e4 scaled by 2^CS_SH, V is
# packed in rank pairs as fp8 scaled by 2^VD_SH; the tail evac rescales
# by 2^-(CS_SH+VD_SH).  gdiag is pre-scaled by the same amount on host.
# The dynamic path is ~0.6% of the output's magnitude, so fp8's ~3%
# relative error contributes ~2e-4 of output scale.
BAND_SH = 7  # band ships as fp8e4 scaled 2^BAND_SH (halves the 9MB DMA)
CS_SH = 7  # equals BAND_SH: cs inherits the band scaling through the matmul
VD_SH = 5


def _dve_taps(h):
    return PE_TAPS.get(h, 0), ACT_TAPS.get(h, (KS[h],))[0]


_MODULE_CACHE = {}


def _install_ntff_hook_shim():
    """This image's antenv lacks axon_hooks; provide it so profiling works."""
    if "antenv.axon_hooks" in sys.modules:
        return
    try:
        from trn_agent_boot.trn_boot import _ntff_profile_via_ctypes

        hook = _ntff_profile_via_ctypes("/opt/axon/libaxon_pjrt.so")
    except Exception:
        hook = None
    mod = types.ModuleType("antenv.axon_hooks")
    mod.get_axon_ntff_profile_hook = lambda: hook
    mod.set_axon_ntff_profile_hook = lambda h: None
    sys.modules["antenv.axon_hooks"] = mod


def _split_multi_waits(nc, mybir):
    """walrus codegen allows a single sync-wait per instruction; hoist
    extras onto a chain of single-wait NoOps on the same engine."""
    for f in nc.m.functions:
        for blk in f.blocks:
            new_insts = []
            for inst in blk.instructions:
                si = getattr(inst, "sync_info", None)
                ow = list(si.on_wait) if si and si.on_wait else []
                if len(ow) >= 2:
                    for i, w in enumerate(ow[:-1]):
                        new_insts.append(
                            mybir.InstNoOp(
                                name=f"{inst.name}-wn{i}",
                                ins=[],
                                outs=[],
                                engine=inst.engine,
                                sync_info=mybir.SyncInfo(on_wait=[w], on_update=[]),
                            )
                        )
                    inst.sync_info = mybir.SyncInfo(
                        on_wait=[ow[-1]],
                        on_update=list(si.on_update) if si.on_update else [],
                    )
                new_insts.append(inst)
            blk.instructions = new_insts


def _tile_cols(h):
    """Band cols per token-tile for head h: C (R*128) + L (R*p) + R (R*p)."""
    p = KS[h] // 2
    return R * (128 + 2 * p)


def _band_off(h):
    """Column offset of head h's packed per-tile band blocks."""
    off = 0
    for g in range(h):
        off += NT * _tile_cols(g)
    return off


BAND_TOTAL = _band_off(H - 1) + NT * _tile_cols(H - 1)

GD_OFF = {}
_o = 0
for _h in sorted(PE_TAPS):
    GD_OFF[_h] = _o
    _o += PE_TAPS[_h] * DH
GD_TOTAL = _o


def _build_module(has_bias):
    import concourse.bass as bass
    import concourse.tile as tile
    from concourse import mybir

    f32 = mybir.dt.float32
    bf16 = mybir.dt.bfloat16
    MULT = mybir.AluOpType.mult
    ADD = mybir.AluOpType.add
    IDENT = mybir.ActivationFunctionType.Identity

    nc = bass.Bass(trn_type="TRN2")

    # host ships these pre-interleaved in SBUF layout: contiguous 2KB+
    # rows keep HWDGE descriptor counts at 128/transfer (a [p, i, c]
    # block-strided AP costs ~1024 descriptors = ~3-11us of engine time)
    xT_d = nc.dram_tensor("xT", [128, H * TH], bf16, kind="ExternalInput")
    w_inT_d = nc.dram_tensor("w_inT", [128, H * DM], bf16, kind="ExternalInput")
    w_outT_d = nc.dram_tensor("w_outT", [128, H * DM], bf16, kind="ExternalInput")
    band_d = nc.dram_tensor(
        "band", [128, BAND_TOTAL], mybir.dt.float8e4, kind="ExternalInput"
    )
    vdiag_d = nc.dram_tensor(
        "vdiag", [DH, H * R * DH], mybir.dt.float8e4, kind="ExternalInput"
    )
    gdiag_d = nc.dram_tensor("gdiag", [DH, GD_TOTAL], bf16, kind="ExternalInput")
    gvec_d = nc.dram_tensor("gvec", [DH, H * 21], f32, kind="ExternalInput")
    b_in_d = nc.dram_tensor("b_in", [128, H], f32, kind="ExternalInput")
    if has_bias:
        b_out_d = nc.dram_tensor("b_out", [1, DM], bf16, kind="ExternalInput")
    out_d = nc.dram_tensor("out", [TPC, DM], bf16, kind="ExternalOutput")


    with tile.TileContext(nc) as tc:
        with tc.tile_pool(name="const", bufs=1) as pc:
            xp_sb = [pc.tile([DH, XF], bf16, name=f"xp{m}") for m in range(H)]
            o_sb = [pc.tile([DH, TPC], bf16, name=f"o{h}") for h in range(H)]
            w_all = pc.tile([128, H * DM], bf16, name="w_all")
            xT_all = pc.tile([128, H * TH], bf16, name="xT_all")
            wo_all = pc.tile([128, H * DM], bf16, name="wo_all")
            gvec_sb = pc.tile([DH, H * 21], f32, name="gvec_sb")
            vd_sb = pc.tile([DH, H * R * DH], mybir.dt.float8e4, name="vd_sb")
            if GD_TOTAL:
                gd_sb = pc.tile([DH, GD_TOTAL], bf16, name="gd_sb")
            b_in_sb = pc.tile([128, H], f32, name="b_in_sb")
            if has_bias:
                ones_sb = pc.tile([1, 128], bf16, name="ones_sb")
                bo_sb = pc.tile([1, DM], bf16, name="bo_sb")
                nc.gpsimd.memset(ones_sb, 1.0)
                nc.scalar.dma_start(out=bo_sb, in_=b_out_d[:, :])


            for m in range(H):
                nc.gpsimd.memset(xp_sb[m][:, 0 : LP - PAD], 0)
                nc.gpsimd.memset(xp_sb[m][:, LP + TPC + PAD : XF], 0)

            # warm the activation table off the critical path
            warm = pc.tile([1, 2], f32, name="warm")
            nc.gpsimd.memset(warm, 0)
            nc.scalar.activation(
                out=warm[:, 1:2], in_=warm[:, 0:1], func=IDENT, bias=0.0, scale=1.0
            )

            # ---- preamble DMAs, first-needed first ----
            # w is m-major [128, (m, i*128)], xT is [128, (i, c)]; all
            # transfers are contiguous-row (128 descriptors).  Act queue:
            # b_in + w blocks; SP queue: xT halves + transposes; Pool
            # queue: bands + cold weights.
            m0, m1 = HEADS[0], HEADS[1]
            MB = 8 * 128  # cols per m-block in w_all
            nc.scalar.dma_start(
                out=w_all[:, m0 * MB : (m0 + 1) * MB],
                in_=w_inT_d[:, m0 * MB : (m0 + 1) * MB],
            )
            # xT rides the Act queue: q1 (SP) stays clear for the
            # latency-critical per-group transposes
            def xt_chunk(ci, eng):
                o0, cn = CH_OFF[ci], S1CH[ci][1]
                eng.dma_start(
                    out=xT_all[:, o0 : o0 + H * cn], in_=xT_d[:, o0 : o0 + H * cn]
                )

            xt_chunk(0, nc.scalar)
            nc.scalar.dma_start(out=b_in_sb, in_=b_in_d[:, :])
            xt_chunk(1, nc.scalar)
            xt_chunk(2, nc.scalar)
            nc.scalar.dma_start(
                out=w_all[:, m1 * MB : (m1 + 1) * MB],
                in_=w_inT_d[:, m1 * MB : (m1 + 1) * MB],
            )
            nc.scalar.dma_start(out=gvec_sb, in_=gvec_d[:, :])

            with tc.tile_pool(name="ps1", bufs=2, space="PSUM") as pp1, tc.tile_pool(
                name="ps3", bufs=2, space="PSUM"
            ) as pp3, tc.tile_pool(
                name="pso", bufs=2, space="PSUM"
            ) as pp_o, tc.tile_pool(name="pband", bufs=3) as p_band, tc.tile_pool(
                name="pcs", bufs=3
            ) as p_cs, tc.tile_pool(name="pxtd", bufs=3) as p_xtd, tc.tile_pool(
                name="pchain", bufs=2
            ) as p_ch, tc.tile_pool(name="pat", bufs=4) as p_at:
                band_tiles = {}
                cs_tiles = {}
                xtd_tiles = {}
                chain_tiles = {}
                pool_tiles = {}

                def issue_band_dma(h, split=False):
                    boff = _band_off(h)
                    bw = NT * _tile_cols(h)
                    bt = p_band.tile(
                        [128, bw], mybir.dt.float8e4, name=f"band{h}", tag="band"
                    )
                    if split:
                        # first head: land the first pairs' blocks early
                        hw_ = 4 * _tile_cols(h)
                        nc.gpsimd.dma_start(
                            out=bt[:, :hw_], in_=band_d[:, boff : boff + hw_]
                        )
                        nc.gpsimd.dma_start(
                            out=bt[:, hw_:], in_=band_d[:, boff + hw_ : boff + bw]
                        )
                    else:
                        # Pool-issued DMA: separate queue, no FIFO conflict
                        # with the JIT transposes (SP) or weights (Act)
                        nc.gpsimd.dma_start(out=bt, in_=band_d[:, boff : boff + bw])
                    band_tiles[h] = bt

                def s1_chunk(m, ci):
                    c0, cn = S1CH[ci]
                    # 512-wide f32 tile = exactly one PSUM bank (zero-region
                    # aligned); only the first cn cols are used
                    ps1 = pp1.tile([128, 512], f32, name="ps1", tag="ps1")
                    o0 = CH_OFF[ci]
                    for i in range(H):
                        wb = (m * H + i) * 128
                        nc.tensor.matmul(
                            ps1[:, :cn],
                            w_all[:, wb : wb + 128],
                            xT_all[:, o0 + i * cn : o0 + (i + 1) * cn],
                            start=(i == 0),
                            stop=(i == H - 1),
                        )
                    nc.scalar.activation(
                        out=xp_sb[m][:, LP - PAD + c0 : LP - PAD + c0 + cn],
                        in_=ps1[:, :cn],
                        func=IDENT,
                        bias=b_in_sb[:, m : m + 1],
                        scale=1.0,
                    )

                def issue_transpose_0(m):
                    # tiles 0..6 (xp cols 0:896) — gated on s1 chunks 0-1;
                    # covers band pairs 0..2 of the next group
                    xtd = p_xtd.tile([128, XF // 128, 128], bf16, name="xtd", tag="xtd")
                    nc.sync.dma_start_transpose(
                        out=xtd[:, 0:7, :], in_=xp_sb[m][:, 0:896]
                    )
                    xtd_tiles[m] = xtd

                def issue_transpose_2(m):
                    nc.sync.dma_start_transpose(
                        out=xtd_tiles[m][:, 7:10, :], in_=xp_sb[m][:, 896:XF]
                    )

                def chain_cont(eng, tile_out, taps):
                    for in0, sc in taps:
                        eng.scalar_tensor_tensor(
                            out=tile_out,
                            in0=in0,
                            scalar=sc,
                            in1=tile_out,
                            op0=MULT,
                            op1=ADD,
                        )

                def chain(eng, tile_out, taps):
                    in0, sc = taps[0]
                    eng.tensor_scalar(
                        out=tile_out, in0=in0, scalar1=sc, scalar2=None, op0=MULT
                    )
                    chain_cont(eng, tile_out, taps[1:])

                def band_pair(h, pair):
                    """Band matmuls for token tiles 2*pair, 2*pair+1 into one
                    2-bank PSUM tile + a single paired Act evac.  Emission is
                    stationary-grouped so each xtd tile is loaded once."""
                    k = KS[h]
                    p = k // 2
                    tcols = _tile_cols(h)
                    bt = band_tiles[h]
                    xtd = xtd_tiles[h]
                    if pair == 0:
                        cs = p_cs.tile(
                            [128, R, TPC], mybir.dt.float8e4, name=f"cs{h}", tag="cs"
                        )
                        cs_tiles[h] = cs
                    cs = cs_tiles[h]
                    psp = pp3.tile([128, 2, R, 128], f32, name="ps_s", tag="ps_s")

                    def blocks(b):
                        o = b * tcols
                        bC = bt[:, o : o + R * 128].rearrange("q (r w) -> q r w", r=R)
                        bL = bt[:, o + R * 128 : o + R * 128 + R * p].rearrange(
                            "q (r w) -> q r w", r=R
                        )
                        bR = bt[:, o + R * 128 + R * p : o + tcols].rearrange(
                            "q (r w) -> q r w", r=R
                        )
                        return bC, bL, bR

                    b0 = 2 * pair
                    b1 = b0 + 1
                    C0, L0, R0 = blocks(b0)
                    C1, L1, R1 = blocks(b1)
                    ps0 = psp[:, 0]
                    ps1_ = psp[:, 1]
                    # stationary sequence: b0 | b0+1 (L1, C0) | b0+2 (C1, R0) | b0+3
                    nc.tensor.matmul(
                        ps0[:, :, 0:p], xtd[:, b0, :], L0, start=True, stop=False
                    )
                    nc.tensor.matmul(
                        ps1_[:, :, 0:p], xtd[:, b1, :], L1, start=True, stop=False
                    )
                    nc.tensor.matmul(ps0, xtd[:, b0 + 1, :], C0, start=False, stop=False)
                    nc.tensor.matmul(
                        ps1_, xtd[:, b1 + 1, :], C1, start=False, stop=False
                    )
                    nc.tensor.matmul(
                        ps0[:, :, 128 - p : 128],
                        xtd[:, b0 + 2, :],
                        R0,
                        start=False,
                        stop=True,
                    )
                    nc.tensor.matmul(
                        ps1_[:, :, 128 - p : 128],
                        xtd[:, b1 + 2, :],
                        R1,
                        start=False,
                        stop=True,
                    )
                    nc.scalar.copy(
                        cs[:, :, b0 * 128 : (b0 + 2) * 128],
                        psp.rearrange("q b r w -> q r b w"),
                    )
                    if pair == 3:
                        band_tiles.pop(h)
                        xtd_tiles.pop(h)

                def tap(h, j):
                    p = KS[h] // 2
                    return (
                        xp_sb[h][:, LP + j - p : LP + j - p + TPC],
                        gvec_sb[:, h * 21 + j : h * 21 + j + 1],
                    )

                def emit_chain_steps(steps, n):
                    for _ in range(n):
                        eng_op = steps.pop(0)
                        eng_op()

                def build_chain_steps(h, tiles, taps):
                    # round-robin taps over the sub-chain tiles; consecutive
                    # DVE ops then target different tiles and pipeline
                    nt = len(tiles)
                    steps = []
                    for i, (in0, sc) in enumerate(taps):
                        t = tiles[i % nt]
                        if i < nt:
                            steps.append(
                                lambda t=t, in0=in0, sc=sc: nc.vector.tensor_scalar(
                                    out=t, in0=in0, scalar1=sc, scalar2=None, op0=MULT
                                )
                            )
                        else:
                            steps.append(
                                lambda t=t, in0=in0, sc=sc: nc.vector.scalar_tensor_tensor(
                                    out=t,
                                    in0=in0,
                                    scalar=sc,
                                    in1=t,
                                    op0=MULT,
                                    op1=ADD,
                                )
                            )
                    return steps

                def band_chains(h):
                    # static MAC chains for taps not on PE, split: first half
                    # here, second half in tail_stage's group so long chains
                    # don't pin the tail's PSUM banks for a whole group.
                    # ACT_TAPS are computed as Act per-partition-scaled
                    # multiplies; the DVE only pays a 2x-rate add for them.
                    j0, j1 = _dve_taps(h)
                    taps = [tap(h, j) for j in range(j0, j1)]
                    if not taps:
                        return
                    tiles = [p_ch.tile([DH, TPC], bf16, name=f"ch{h}", tag="chA")]
                    if len(taps) >= 6:
                        tiles.append(
                            p_ch.tile([DH, TPC], bf16, name=f"ch{h}b", tag="chB")
                        )
                    steps = build_chain_steps(h, tiles, taps)
                    if h in ACT_TAPS:
                        q0, q1 = ACT_TAPS[h]
                        asteps = []
                        for qi, j in enumerate(range(q0, q1)):
                            in0, gc = tap(h, j)
                            at = p_at.tile([DH, TPC], bf16, name=f"at{h}", tag="at")
                            nc.scalar.mul(at, in0, gc)
                            t = tiles[(len(taps) + qi) % len(tiles)]
                            asteps.append(
                                lambda t=t, at=at: nc.vector.tensor_tensor(
                                    t, t, at, op=ADD
                                )
                            )
                        # act-tap adds run after the sub-chain inits
                        # (the first step of each tile overwrites it)
                        steps = steps + asteps
                    emit_chain_steps(steps, (len(steps) + 1) // 2)
                    chain_tiles[h] = (tiles, steps)

                def tail_stage(h):
                    cs = cs_tiles.pop(h)
                    k = KS[h]
                    p = k // 2
                    j0 = PE_TAPS.get(h, 0)
                    sacc = None
                    if h in chain_tiles:
                        tiles, steps = chain_tiles.pop(h)
                        emit_chain_steps(steps, len(steps))
                        if len(tiles) == 2:
                            nc.vector.tensor_tensor(tiles[0], tiles[0], tiles[1], op=ADD)
                        sacc = tiles[0]
                    tmp_o = p_ch.tile([DH, TPC], bf16, name="tmp_o", tag="tmpo")
                    for ci, c0 in enumerate((0, 512)):
                        ps_o = pp_o.tile([128, 512], f32, name="ps_o", tag="ps_o")
                        n_mm = R // 2 + j0
                        idx = 0
                        for q in range(R // 2):
                            # fp8 DoubleRow: ranks (2q, 2q+1) in one matmul
                            vq = vd_sb[
                                :, (h * R + 2 * q) * DH : (h * R + 2 * q + 2) * DH
                            ].rearrange("d (i e) -> d i e", i=2)
                            nc.tensor.matmul(
                                ps_o,
                                vq,
                                cs[:, 2 * q : 2 * q + 2, c0 : c0 + 512],
                                start=(idx == 0),
                                stop=(idx == n_mm - 1),
                                perf_mode=mybir.MatmulPerfMode.DoubleRow,
                            )
                            idx += 1
                        go = GD_OFF.get(h, 0)
                        for j in range(j0):
                            nc.tensor.matmul(
                                ps_o,
                                gd_sb[:, go + j * DH : go + (j + 1) * DH],
                                xp_sb[h][:, LP + j - p + c0 : LP + j - p + c0 + 512],
                                start=False,
                                stop=(idx == n_mm - 1),
                            )
                            idx += 1
                        # fast Act evac (rescales the 2^(CS_SH+VD_SH) fp8
                        # scaling away) so the PSUM bank frees quickly; the
                        # bf16 chain merge then runs at DVE 2x rate
                        dst = tmp_o if sacc is not None else o_sb[h]
                        nc.scalar.mul(
                            dst[:, c0 : c0 + 512], ps_o, 2.0 ** -(CS_SH + VD_SH)
                        )
                    if sacc is not None:
                        nc.vector.tensor_tensor(o_sb[h], tmp_o, sacc, op=ADD)

                # ---------------- pipelined emission ----------------
                # PE dummy-fill on zeroed data: opens the HAM clock gate and
                # keeps PE busy while the first input DMAs land
                wscr = pc.tile([128, 512], bf16, name="wscr")
                nc.vector.memset(wscr, 0)
                for _ in range(N_WARM):
                    psw = pp1.tile([128, 512], f32, name="ps1", tag="ps1")
                    nc.tensor.matmul(
                        psw, wscr[:, 0:128], wscr, start=True, stop=True
                    )

                issue_band_dma(HEADS[0], split=True)
                # rest of w_in (m-blocks 0..5; m0, m1 are 6 and 7)
                nc.gpsimd.dma_start(
                    out=w_all[:, : 6 * MB], in_=w_inT_d[:, : 6 * MB]
                )
                nc.gpsimd.dma_start(out=vd_sb, in_=vdiag_d[:, :])
                if GD_TOTAL:
                    nc.gpsimd.dma_start(out=gd_sb, in_=gdiag_d[:, :])
                for gi, m in enumerate(HEADS):
                    hp = HEADS[gi - 1] if gi >= 1 else None  # band stage
                    hq = HEADS[gi - 2] if gi >= 2 else None  # tail stage
                    if gi == 3:
                        nc.gpsimd.dma_start(out=wo_all, in_=w_outT_d[:, :])
                    s1_chunk(m, 0)
                    if hp is not None:
                        band_pair(hp, 0)
                    s1_chunk(m, 1)
                    issue_transpose_0(m)
                    if hp is not None:
                        band_pair(hp, 1)
                    s1_chunk(m, 2)
                    issue_transpose_2(m)
                    # next head's band lands behind this group's transpose on
                    # the SP queue: ~1.5 groups of lead, transpose not delayed
                    if gi + 1 < H:
                        issue_band_dma(HEADS[gi + 1])
                    if hp is not None:
                        band_pair(hp, 2)
                        band_pair(hp, 3)
                    if hq is not None:
                        tail_stage(hq)
                    if hp is not None:
                        band_chains(hp)
                for pair in range(4):
                    band_pair(HEADS[7], pair)
                tail_stage(HEADS[6])
                band_chains(HEADS[7])
                tail_stage(HEADS[7])

            # ---------------- stage 4: out projection ----------------
            with tc.tile_pool(name="ps4", bufs=4, space="PSUM") as pp4, tc.tile_pool(
                name="post", bufs=4
            ) as p_ost:
                for t in range(NT):
                    for ei, e0 in enumerate((0, 512)):
                        ps4 = pp4.tile([128, 512], f32, name="ps4", tag="ps4")
                        n_mm = H + (1 if has_bias else 0)
                        for i in range(H):
                            nc.tensor.matmul(
                                ps4,
                                o_sb[i][:, t * 128 : (t + 1) * 128],
                                wo_all[:, i * DM + e0 : i * DM + e0 + 512],
                                start=(i == 0),
                                stop=(i == n_mm - 1),
                            )
                        if has_bias:
                            nc.tensor.matmul(
                                ps4,
                                ones_sb,
                                bo_sb[:, e0 : e0 + 512],
                                start=False,
                                stop=True,
                            )
                        ost = p_ost.tile([128, 512], bf16, name="ost", tag="ost")
                        nc.vector.tensor_scalar(
                            out=ost, in0=ps4, scalar1=1.0, scalar2=None, op0=MULT
                        )
                        eng = nc.sync
                        eng.dma_start(
                            out=out_d[t * 128 : (t + 1) * 128, e0 : e0 + 512],
                            in_=ost,
                        )

    _split_multi_waits(nc, mybir)
    return nc


def _band_bases(A):
    """Per-head unscaled band blocks (f32): C (128,R,128), L/R (128,R,p)."""
    bases = []
    t = np.arange(128)[:, None]
    for h in range(H):
        k = KS[h]
        p = k // 2
        w = np.arange(128)[None, :]
        dC = t - w
        mC = np.abs(dC) <= p
        iC = np.clip(dC + p, 0, k - 1)
        wl = np.arange(p)[None, :] if p else np.zeros((1, 0), int)
        dL = t - wl - 128
        mL = (dL >= -p) & (dL <= p)
        iL = np.clip(dL + p, 0, k - 1)
        u = np.arange(p)[None, :] if p else np.zeros((1, 0), int)
        dR = t + p - u  # t - (128-p+u) + 128
        mR = (dR >= -p) & (dR <= p)
        iR = np.clip(dR + p, 0, k - 1)
        C = np.where(mC[:, None, :], A[h][:, iC].transpose(1, 0, 2), 0.0)
        L = np.where(mL[:, None, :], A[h][:, iL].transpose(1, 0, 2), 0.0)
        Rb = np.where(mR[:, None, :], A[h][:, iR].transpose(1, 0, 2), 0.0)
        bases.append((C, L, Rb))
    return bases


def _host_prep(inputs):
    x = np.ascontiguousarray(np.asarray(inputs["x"], dtype=np.float32))
    W_in = np.asarray(inputs["W_in"], dtype=np.float32)
    b_in = np.asarray(inputs["b_in"], dtype=np.float32)
    W_out = np.asarray(inputs["W_out"], dtype=np.float32)
    b_out = np.asarray(inputs["b_out"], dtype=np.float32)
    Wc = np.asarray(inputs["Wc"], dtype=np.float32)
    A = np.asarray(inputs["A"], dtype=np.float32)
    V = np.asarray(inputs["V"], dtype=np.float32)
    base = np.asarray(inputs["base"], dtype=np.float32)
    alphas = np.asarray(inputs["alphas"], dtype=np.float32)

    alpha = 1.0 / (1.0 + np.exp(-alphas))
    W_inT = np.ascontiguousarray(W_in.T)
    W_outT = np.ascontiguousarray(W_out.T)
    Wc_aug = np.zeros((DM, H * R), dtype=np.float32)
    for h in range(H):
        # alpha folded into c
        Wc_aug[:, R * h : R * h + R] = alpha[h] * (
            W_inT[:, h * DH : (h + 1) * DH] @ Wc[h]
        )

    bases = _band_bases(A)

    gvec = np.zeros((DH, H, 21), dtype=np.float32)
    for h in range(H):
        k = KS[h]
        gvec[:, h, :k] = ((1.0 - alpha[h]) * base[h, :k]).T

    dd = np.arange(DH)
    vd = np.zeros((DH, H, R, DH), dtype=np.float32)
    for h in range(H):
        for r in range(R):
            vd[dd, h, r, dd] = V[h, r]
    gd = np.zeros((DH, max(GD_TOTAL, 1)), dtype=np.float32)
    for h in sorted(PE_TAPS):
        go = GD_OFF[h]
        g = (1.0 - alpha[h]) * base[h, : PE_TAPS[h]]  # (j0, DH)
        for j in range(PE_TAPS[h]):
            gd[dd, go + j * DH + dd] = g[j]

    # pre-interleave into the exact SBUF layouts (contiguous rows keep
    # DMA descriptor counts minimal): w m-major [128, (m, i, 128)],
    # w_out i-major [128, (i, 1024)]
    w_m = (
        W_inT.reshape(H, 128, H, 128).transpose(1, 2, 0, 3).reshape(128, H * DM)
    )
    wo_i = W_outT.reshape(H, 128, DM).transpose(1, 0, 2).reshape(128, H * DM)
    prep = {
        "w_inT": np.ascontiguousarray(w_m).astype(BF16),
        "w_outT": np.ascontiguousarray(wo_i).astype(BF16),
        "vdiag": (vd.reshape(DH, H * R * DH) * 2.0**VD_SH).astype(FP8),
        "gvec": gvec.reshape(DH, H * 21).copy(),
        "b_in": np.ascontiguousarray(b_in.reshape(H, 128).T),
    }
    if GD_TOTAL:
        prep["gdiag"] = (gd[:, :GD_TOTAL] * 2.0 ** (CS_SH + VD_SH)).astype(BF16)
    has_bias = bool(np.any(b_out != 0.0))
    if has_bias:
        prep["b_out"] = b_out.reshape(1, DM).astype(BF16)

    xT_slices = []
    band_slices = []
    per_b = NC // B
    for c in range(NC):
        bb = c // per_b
        s = (c % per_b) * TPC
        sl = np.zeros((TH, DM), dtype=np.float32)
        lo, hi = s - PAD, s + TPC + PAD
        clo, chi = max(lo, 0), min(hi, N)
        sl[clo - lo : chi - lo] = x[bb, clo:chi]
        slT = sl.T.reshape(H, 128, TH).transpose(1, 0, 2)  # [128, i, c]
        xh = np.empty((128, H * TH), dtype=np.float32)
        for ci, (c0, cn) in enumerate(S1CH):
            o0 = CH_OFF[ci]
            xh[:, o0 : o0 + H * cn] = slT[:, :, c0 : c0 + cn].reshape(128, H * cn)
        xT_slices.append(np.ascontiguousarray(xh).astype(BF16))
        cc = (sl[PAD : PAD + TPC] @ Wc_aug).T.reshape(H, R, TPC)  # alpha*c

        band = np.empty((128, BAND_TOTAL), dtype=np.float32)
        for h in range(H):
            k = KS[h]
            p = k // 2
            C, L, Rb = bases[h]
            tcols = _tile_cols(h)
            boff = _band_off(h)
            ch = cc[h]  # (R, TPC)
            for b in range(NT):
                o = boff + b * tcols
                cw = ch[None, :, b * 128 : (b + 1) * 128]  # (1, R, 128)
                band[:, o : o + R * 128] = (C * cw).reshape(128, R * 128)
                if p:
                    cl = ch[None, :, b * 128 : b * 128 + p]
                    band[:, o + R * 128 : o + R * 128 + R * p] = (L * cl).reshape(
                        128, R * p
                    )
                    cr = ch[None, :, (b + 1) * 128 - p : (b + 1) * 128]
                    band[:, o + R * 128 + R * p : o + tcols] = (Rb * cr).reshape(
                        128, R * p
                    )
        band_slices.append((band * 2.0**BAND_SH).astype(FP8))
    return prep, xT_slices, band_slices, has_bias


def _run(inputs, trace=False, **kwargs):
    _install_ntff_hook_shim()
    from concourse.bass_utils import run_bass_kernel_spmd

    prep, xT_slices, band_slices, has_bias = _host_prep(inputs)
    key = ("mod", has_bias)
    if key not in _MODULE_CACHE:
        _MODULE_CACHE[key] = _build_module(has_bias)
    nc = _MODULE_CACHE[key]

    in_maps = []
    for c in range(NC):
        m = dict(prep)
        m["xT"] = xT_slices[c]
        m["band"] = band_slices[c]
        in_maps.append(m)

    res = run_bass_kernel_spmd(
        nc, in_maps, core_ids=list(range(NC)), trace=trace, **kwargs
    )
    outs = [res.results[c]["out"] for c in range(NC)]
    full = np.concatenate(outs, axis=0).reshape(B, N, DM).astype(np.float32)
    return full, res


def kernel(**inputs) -> np.ndarray:
    return _run(inputs)[0]
